# revision 2
# baseline (speedup 1.0000x reference)
"""CNSN (eval-mode CrossNorm+SelfNorm) Trainium2 kernel.

Reference computation (per sample b, channel c over spatial HW):
    mean, std  (unbiased std over the 4096 spatial elements)
    gate_m = sigmoid(MLP_m([mean, std]))      # Linear(2,16)+ReLU+Linear(16,1)
    gate_s = sigmoid(MLP_s([mean, std]))
    out = (x - m)/s * (s*gate_s) + m*gate_m
        = x * gate_s + m * (gate_m - gate_s)   # per-channel affine

Strategy: pure data-parallel over batch (64 samples -> 8 per core).
Per core: 16 tiles of [128 channels, 4096 spatial] f32 (2 MiB each).
Per tile: bn_stats/bn_aggr (DVE) -> tiny fused MLP (DVE+ACT) ->
single ACT activation applies the per-channel affine -> DMA out.
Memory-bound: 32 MiB in + 32 MiB out per core.
"""

import numpy as np

import concourse.bass as bass
import concourse.tile as tile
from concourse import mybir
from concourse.bass_utils import run_bass_kernel_spmd

F32 = mybir.dt.float32
AF = mybir.ActivationFunctionType
ALU = mybir.AluOpType

N_CORES = 8
B, C, H, W = 64, 256, 64, 64
HW = H * W                     # 4096
B_PER_CORE = B // N_CORES      # 8
TILES = B_PER_CORE * C // 128  # 16 tiles of [128, HW] per core
EPS = 1e-5
# bn_aggr returns population variance (M2/n); torch-style unbiased var is
# M2/(n-1), so std = sqrt(var_pop * n/(n-1) + eps).
VAR_CORR = HW / (HW - 1)

# consts layout, one [128, 130] f32 tensor (all rows identical):
#   [:,   0: 32] W10  = concat(wm1, ws1)[:, 0]   (weight on the mean input)
#   [:,  32: 64] W11  = concat(wm1, ws1)[:, 1]   (weight on the std input)
#   [:,  64: 96] B1   = concat(bm1, bs1)
#   [:,  96:112] W2M  = wm2[0]
#   [:, 112:128] W2S  = ws2[0]
#   [:, 128:129] B2M  = bm2[0]
#   [:, 129:130] B2S  = bs2[0]
N_CONST = 130

_CACHE: dict = {}
LAST_RESULTS = None  # BassKernelResults of the most recent run (for profiling)


def _split_excess_waits(nc: bass.Bass) -> int:
    """Move surplus sync waits onto standalone nops.

    The TPB EVENTS field encodes exactly ONE wait per hardware instruction
    (see NEURON_ISA_TPB_EVENTS); walrus codegen hard-fails with "Too many
    sync wait commands" when Tile attaches more. Sequencers execute
    same-engine instructions in program order, so hoisting all but one wait
    onto nofuse nops placed immediately before the instruction preserves
    semantics.
    """
    builder_of = {
        mybir.EngineType.DVE: nc.vector,
        mybir.EngineType.Activation: nc.scalar,
        mybir.EngineType.PE: nc.tensor,
        mybir.EngineType.Pool: nc.gpsimd,
        mybir.EngineType.SP: nc.sync,
    }
    n_split = 0
    for bb in nc.main_func.blocks:
        insts = bb.instructions
        out = []
        changed = False
        for ins in list(insts):
            si = ins.sync_info
            if si is not None and si.on_wait and len(si.on_wait) > 1:
                assert si.on_update is None or len(si.on_update) <= 1, ins
                waits = list(si.on_wait)
                for w in waits[:-1]:
                    nop = builder_of[ins.engine].nop(nofuse=True).ins
                    # the builder appended it to some (current) block; yank it
                    for b2 in nc.main_func.blocks:
                        try:
                            b2.instructions.remove(nop)
                            break
                        except ValueError:
                            pass
                    nop.sync_info = mybir.SyncInfo(on_wait=[w], on_update=[])
                    out.append(nop)
                ins.sync_info = mybir.SyncInfo(
                    on_wait=[waits[-1]], on_update=list(si.on_update or [])
                )
                changed = True
                n_split += 1
            out.append(ins)
        if changed:
            insts.clear()
            insts.extend(out)
    return n_split


def _build_nc(repeat: int = 1) -> bass.Bass:
    """Build the per-core Bass program.

    repeat > 1 (odd) chains N tile sweeps inside one NEFF, each reading the
    previous sweep's output (x -> y -> scratch -> y -> ...), so no sweep's
    traffic can be elided or overlapped away. Used only by timing.py:
    per-sweep HW time = slope of wall time between two repeat values, which
    cancels the multi-ms axon dispatch overhead that would otherwise swamp
    a sub-millisecond kernel.
    """
    assert repeat % 2 == 1, "odd repeat keeps the final sweep writing y"
    nc = bass.Bass()
    x = nc.declare_dram_parameter("x", [TILES, 128, HW], F32, isOutput=False)
    cn = nc.declare_dram_parameter("consts", [128, N_CONST], F32, isOutput=False)
    y = nc.declare_dram_parameter("y", [TILES, 128, HW], F32, isOutput=True)
    scratch = nc.dram_tensor("scratch", [TILES, 128, HW], F32) if repeat > 1 else None

    with tile.TileContext(nc) as tc:
        with (
            tc.tile_pool(name="consts", bufs=1) as consts,
            tc.tile_pool(name="xin", bufs=5) as xin,
            tc.tile_pool(name="yout", bufs=4) as yout,
            tc.tile_pool(name="small", bufs=6) as small,
        ):
            cst0 = consts.tile([128, N_CONST], F32)
            nc.sync.dma_start(out=cst0[:], in_=cn[:, :])
            # Bounce through DVE so every DVE consumer of the constants
            # depends on a same-engine product: the consts-DMA wait then
            # lives on this copy (TensorCopy has spare sync-wait slots)
            # instead of a TensorScalarPtr, whose encoding has only one.
            cst = consts.tile([128, N_CONST], F32)
            nc.vector.tensor_copy(out=cst[:], in_=cst0[:])
            eps_t = consts.tile([128, 1], F32)
            nc.vector.memset(eps_t[:], EPS)
            w10t = cst[:, 0:32]
            w11t = cst[:, 32:64]
            b1t = cst[:, 64:96]
            w2mt = cst[:, 96:112]
            w2st = cst[:, 112:128]
            b2mt = cst[:, 128:129]
            b2st = cst[:, 129:130]

            for r in range(repeat):
              src = x if r == 0 else (y if r % 2 == 1 else scratch)
              dst = y if r % 2 == 0 else scratch
              for i in range(TILES):
                xt = xin.tile([128, HW], F32)
                nc.sync.dma_start(out=xt[:], in_=src[i, :, :])
                yt = yout.tile([128, HW], F32)

                # ACT pre-touches: absorb the x-load DMA wait and the y-slot
                # store-WAR DMA wait on dedicated Copy ops. Without these the
                # apply activation below needs 3-4 sync waits, which exceeds
                # the Activation ISA encoding's 2 wait slots (walrus "Too
                # many sync wait commands").
                pre = small.tile([128, 1], F32)
                nc.scalar.activation(out=pre[:], in_=xt[:, 0:1], func=AF.Copy)
                nc.scalar.activation(out=yt[:, 0:1], in_=pre[:], func=AF.Copy)

                # mean / population-variance over the free axis
                stats = small.tile([128, HW // 512, nc.vector.BN_STATS_DIM], F32)
                xv = xt[:].rearrange("p (a b) -> p a b", b=512)
                for s in range(HW // 512):
                    nc.vector.bn_stats(out=stats[:, s, :], in_=xv[:, s, :])
                mv = small.tile([128, nc.vector.BN_AGGR_DIM], F32)
                nc.vector.bn_aggr(out=mv[:], in_=stats[:])
                mean = mv[:, 0:1]

                # std = sqrt(var_pop * n/(n-1) + eps)
                sd = small.tile([128, 1], F32)
                nc.scalar.activation(
                    out=sd[:], in_=mv[:, 1:2], func=AF.Sqrt, bias=eps_t[:],
                    scale=VAR_CORR,
                )

                # layer 1 (both MLPs fused, 32 hidden units total):
                # h = relu(mean*W10 + std*W11 + B1)
                t1 = small.tile([128, 32], F32)
                nc.vector.tensor_scalar_mul(out=t1[:], in0=w10t, scalar1=mean)
                t2 = small.tile([128, 32], F32)
                nc.vector.tensor_scalar_mul(out=t2[:], in0=w11t, scalar1=sd[:])
                h = small.tile([128, 32], F32)
                nc.vector.tensor_add(out=h[:], in0=t1[:], in1=t2[:])
                nc.vector.tensor_add(out=h[:], in0=h[:], in1=b1t)
                nc.vector.tensor_scalar_max(out=h[:], in0=h[:], scalar1=0.0)

                # layer 2: gate = sigmoid(h . w2 + b2), per branch
                hw2 = small.tile([128, 32], F32)
                nc.vector.tensor_mul(out=hw2[:], in0=h[:], in1=cst[:, 96:128])
                gm = small.tile([128, 1], F32)
                nc.vector.reduce_sum(
                    out=gm[:], in_=hw2[:, 0:16], axis=mybir.AxisListType.X
                )
                gs = small.tile([128, 1], F32)
                nc.vector.reduce_sum(
                    out=gs[:], in_=hw2[:, 16:32], axis=mybir.AxisListType.X
                )
                gate_m = small.tile([128, 1], F32)
                nc.scalar.activation(
                    out=gate_m[:], in_=gm[:], func=AF.Sigmoid, bias=b2mt, scale=1.0
                )
                gate_s = small.tile([128, 1], F32)
                nc.scalar.activation(
                    out=gate_s[:], in_=gs[:], func=AF.Sigmoid, bias=b2st, scale=1.0
                )

                # bias_c = (gate_m - gate_s) * mean ; out = gate_s * x + bias_c
                bc = small.tile([128, 1], F32)
                nc.vector.tensor_sub(out=bc[:], in0=gate_m[:], in1=gate_s[:])
                nc.vector.tensor_mul(out=bc[:], in0=bc[:], in1=mean)
                nc.scalar.activation(
                    out=yt[:], in_=xt[:], func=AF.Identity, bias=bc[:], scale=gate_s[:]
                )
                # SWDGE (gpsimd) stores use separate DMA queue rows from the
                # HWDGE loads: HW-measured ~145us/sweep vs ~160us with
                # stores on the scalar HWDGE ring.
                nc.gpsimd.dma_start(out=dst[i, :, :], in_=yt[:])
    _split_excess_waits(nc)
    nc.finalize()
    return nc


def _pack_consts(wm1, bm1, wm2, bm2, ws1, bs1, ws2, bs2) -> np.ndarray:
    w1 = np.concatenate([wm1, ws1], axis=0).astype(np.float32)  # [32, 2]
    b1 = np.concatenate([bm1, bs1], axis=0).astype(np.float32)  # [32]
    row = np.concatenate(
        [
            w1[:, 0], w1[:, 1], b1,
            wm2[0].astype(np.float32), ws2[0].astype(np.float32),
            bm2.astype(np.float32).reshape(1), bs2.astype(np.float32).reshape(1),
        ]
    )
    assert row.shape == (N_CONST,)
    return np.ascontiguousarray(np.broadcast_to(row, (128, N_CONST))).astype(np.float32)


def _prep_x_shard(x, core):
    return np.ascontiguousarray(
        x[core * B_PER_CORE : (core + 1) * B_PER_CORE]
    ).reshape(TILES, 128, HW)


def kernel(x, wm1, bm1, wm2, bm2, ws1, bs1, ws2, bs2):
    global LAST_RESULTS
    x = np.asarray(x, dtype=np.float32)
    assert x.shape == (B, C, H, W)
    consts = _pack_consts(wm1, bm1, wm2, bm2, ws1, bs1, ws2, bs2)

    if "nc" not in _CACHE:
        _CACHE["nc"] = _build_nc()
    nc = _CACHE["nc"]

    in_maps = []
    for c in range(N_CORES):
        in_maps.append({"x": _prep_x_shard(x, c), "consts": consts})

    res = run_bass_kernel_spmd(nc, in_maps, list(range(N_CORES)))
    LAST_RESULTS = res
    y = np.concatenate(
        [
            res.results[c]["y"].reshape(B_PER_CORE, C, H, W)
            for c in range(N_CORES)
        ],
        axis=0,
    )
    return np.ascontiguousarray(y, dtype=np.float32)

